# revision 9
# baseline (speedup 1.0000x reference)
"""Trainium2 Bass kernel for fused additive-attention pooling (nn_Attention).

Reference: logits = enc[b] @ w_enc (+const -> drops in softmax); attn =
softmax(logits); out[b] = attn @ enc[b].

All-PE variant: the host stores enc TWICE in fp8 e3m4 (same total bytes as
one fp16 copy):
  encA [l, d] row-major   - context pass
  encB [d, l] row-major   - logits pass
Both heavy passes become 128x128-stationary-weight matmuls with fp8
FastWeightLoad (4 weights/cycle/lane), so the PE does everything
(~16 us/rep) and the DVE is idle. Measured rel_err vs the f32 reference
is 1.2e-2 (tolerance 2e-2); all accumulation is f32 in PSUM, w and p stay
fp16.

Per batch b (P=128, NT=16 l-chunks, DC=8 d-chunks):
  s[lc]   = sum_c encB[c, lc].T @ w[c]      # 8 MMs -> PSUM [128, 1]
  p[:,lc] = exp(s[lc])                      # ACT, fp16
  ctx[c] += encA[lc, c].T @ p[:, lc]        # 8 MMs per lc -> PSUM [128,1]
  z      += p[:, lc].T @ ones               # 1 MM
Host divides ctx/z and reorders d = c*128 + p.

Sharding: data-parallel over batch B=32 across 8 NeuronCores (4/core).
"""

import sys

if "/opt/trn_rl_repo" not in sys.path:
    sys.path.insert(0, "/opt/trn_rl_repo")

import numpy as np
import ml_dtypes

import concourse.bacc as bacc
import concourse.mybir as mybir
import concourse.tile as tile
from concourse import bass_utils

F8NP = ml_dtypes.float8_e3m4

B, L, D = 32, 2048, 1024
NCORES = 8
B_LOC = B // NCORES          # 4 batches per core
P = 128                      # SBUF partitions
NT = L // P                  # 16 l-chunks per batch
DC = D // P                  # 8 d-chunks

KA = 16                      # encA tiles per DMA (16 KiB partition lines)
KB = 8                       # encB chunk-rows per DMA (16 KiB partition lines)
ENCA_BUFS = 4                # encA group slots [128, KA, D] f8 (2 MiB each);
                             # a batch's 2 groups stay live through its ctx
                             # phase (c-outer loop), plus next-batch prefetch
ENCB_BUFS = 4                # encB group slots [128, KB, L] f8 (2 MiB each)
                             # encB rides the ACT HWDGE ring, encA the sync
                             # ring - two descriptor engines drain in parallel


def _build(reps=1):
    nc = bacc.Bacc("TRN2", target_bir_lowering=False, debug=False, num_devices=NCORES)
    f32 = mybir.dt.float32
    f16 = mybir.dt.float16
    f8 = mybir.dt.float8e3
    encA = nc.dram_tensor("encA", [B_LOC * L, D], f8, kind="ExternalInput")
    encB = nc.dram_tensor("encB", [B_LOC * D, L], f8, kind="ExternalInput")
    wcol = nc.dram_tensor("wcol", [P, DC], f16, kind="ExternalInput")
    ctxout = nc.dram_tensor("ctxout", [P, B_LOC * DC], f32, kind="ExternalOutput")
    zout = nc.dram_tensor("zout", [1, B_LOC], f32, kind="ExternalOutput")

    with tile.TileContext(nc) as tc:
        with (
            tc.tile_pool(name="const", bufs=1) as const_pool,
            tc.tile_pool(name="encb", bufs=ENCB_BUFS) as encb_pool,
            tc.tile_pool(name="enca", bufs=ENCA_BUFS) as enca_pool,
            tc.tile_pool(name="pp", bufs=2) as p_pool,
            tc.tile_pool(name="ctxsb", bufs=2) as ctxsb_pool,
            tc.tile_pool(name="zsb", bufs=2) as zsb_pool,
            tc.tile_pool(name="pss", bufs=2, space="PSUM") as ps_s,
            tc.tile_pool(name="psctx", bufs=2, space="PSUM") as ps_ctx,
            tc.tile_pool(name="psz", bufs=2, space="PSUM") as ps_z,
        ):
            wt = const_pool.tile([P, DC], f16)
            nc.sync.dma_start(wt[:], wcol[:])
            ones = const_pool.tile([P, 1], f16)
            nc.vector.memset(ones[:], 1.0)
            warm_in = const_pool.tile([1, 1], f32)
            nc.vector.memset(warm_in[:], 1.0)

            # Cold-start warmups: preload the ACT exp table and keep the PE
            # busy so the HAM clock gate reaches full rate.
            warm = zsb_pool.tile([1, 1], f16)
            nc.scalar.activation(
                warm[:], warm_in[:], mybir.ActivationFunctionType.Exp
            )
            wps = ps_z.tile([1, 1], f32)
            for i in range(48):
                nc.tensor.matmul(wps[:], ones[:], ones[:])

            for _ in range(reps):
                ctx_sb = ctxsb_pool.tile([P, B_LOC * DC], f32)
                z_sb = zsb_pool.tile([1, B_LOC], f32)
                for b in range(B_LOC):
                    # --- logits phase: encB chunks [128_d, L] ---
                    ebufs = []
                    for c0 in range(0, DC, KB):
                        r0 = (b * DC + c0) * P
                        ebuf = encb_pool.tile([P, KB, L], f8)
                        nc.scalar.dma_start(
                            ebuf[:],
                            encB[r0 : r0 + KB * P, :].rearrange(
                                "(k p) l -> p k l", p=P
                            ),
                        )
                        for j in range(KB):
                            ebufs.append(ebuf[:, j, :])
                    p_sb = p_pool.tile([P, NT], f16)
                    for t in range(NT):
                        s_ps = ps_s.tile([P, 1], f32)
                        for c in range(DC):
                            nc.tensor.matmul(
                                s_ps[:],
                                ebufs[c][:, t * P : (t + 1) * P],
                                wt[:, c : c + 1],
                                start=(c == 0),
                                stop=(c == DC - 1),
                                skip_group_check=True,
                            )
                        nc.scalar.activation(
                            p_sb[:, t : t + 1],
                            s_ps[:],
                            mybir.ActivationFunctionType.Exp,
                        )
                    # --- context phase: encA tiles [128_l, D] ---
                    ctx_ps = ps_ctx.tile([P, DC], f32)
                    z_ps = ps_z.tile([1, 1], f32)
                    abufs = []
                    for t0 in range(0, NT, KA):
                        r0 = (b * NT + t0) * P
                        abuf = enca_pool.tile([P, KA, D], f8)
                        nc.sync.dma_start(
                            abuf[:],
                            encA[r0 : r0 + KA * P, :].rearrange(
                                "(k p) d -> p k d", p=P
                            ),
                        )
                        for j in range(KA):
                            abufs.append(abuf[:, j, :])
                    # c-outer / t-inner: each PSUM column's accumulation group
                    # fully closes before the next column's start clears the
                    # bank's has_written bits (start is bank-wide, values
                    # persist).
                    for c in range(DC):
                        for t in range(NT):
                            nc.tensor.matmul(
                                ctx_ps[:, c : c + 1],
                                abufs[t][:, c * P : (c + 1) * P],
                                p_sb[:, t : t + 1],
                                start=(t == 0),
                                stop=(t == NT - 1),
                                skip_group_check=True,
                            )
                    for t in range(NT):
                        nc.tensor.matmul(
                            z_ps[:],
                            p_sb[:, t : t + 1],
                            ones[:],
                            start=(t == 0),
                            stop=(t == NT - 1),
                        )
                    nc.scalar.activation(
                        ctx_sb[:, b * DC : (b + 1) * DC],
                        ctx_ps[:],
                        mybir.ActivationFunctionType.Copy,
                    )
                    nc.scalar.activation(
                        z_sb[:, b : b + 1],
                        z_ps[:],
                        mybir.ActivationFunctionType.Copy,
                    )
                nc.scalar.dma_start(ctxout[:], ctx_sb[:])
                nc.scalar.dma_start(zout[:], z_sb[:])
    nc.compile()
    return nc


_NC = None


def _get_nc():
    global _NC
    if _NC is None:
        _NC = _build()
    return _NC


def _prep_inputs(encoder_output, W):
    enc32 = np.asarray(encoder_output, dtype=np.float32)
    encA = enc32.astype(F8NP)                                    # [B, L, D]
    encB = np.ascontiguousarray(enc32.transpose(0, 2, 1)).astype(F8NP)  # [B, D, L]
    w16 = np.asarray(W, dtype=np.float32)[:D, 0].astype(np.float16)
    wcol = np.ascontiguousarray(w16.reshape(DC, P).T)            # [128, 8]
    return encA, encB, wcol


def _run(nc, encA, encB, wcol, **kwargs):
    in_maps = [
        {
            "encA": np.ascontiguousarray(
                encA[i * B_LOC : (i + 1) * B_LOC].reshape(B_LOC * L, D)
            ),
            "encB": np.ascontiguousarray(
                encB[i * B_LOC : (i + 1) * B_LOC].reshape(B_LOC * D, L)
            ),
            "wcol": wcol,
        }
        for i in range(NCORES)
    ]
    res = bass_utils.run_bass_kernel_spmd(
        nc, in_maps, core_ids=list(range(NCORES)), **kwargs
    )
    out = np.empty((B, 1, D), np.float32)
    for i, r in enumerate(res.results):
        ctx = r["ctxout"]          # [128, B_LOC*DC]
        zz = r["zout"]             # [1, B_LOC]
        for b in range(B_LOC):
            m = ctx[:, b * DC : (b + 1) * DC]       # [128_p, DC_c]
            out[i * B_LOC + b, 0, :] = m.T.reshape(D) / zz[0, b]
    return out, res


def kernel(encoder_output, decoder_hidden=None, W=None, b=None):
    encA, encB, wcol = _prep_inputs(encoder_output, W)
    out, _ = _run(_get_nc(), encA, encB, wcol)
    return out


# revision 11
# speedup vs baseline: 1.3559x; 1.3559x over previous
"""Trainium2 Bass kernel for fused additive-attention pooling (nn_Attention).

Reference: logits = enc[b] @ w_enc (+const -> drops in softmax); attn =
softmax(logits); out[b] = attn @ enc[b].

All-PE variant: the host stores enc TWICE in fp8 e3m4 (same total bytes as
one fp16 copy):
  encA [l, d] row-major   - context pass
  encB [d, l] row-major   - logits pass
Both heavy passes become 128x128-stationary-weight matmuls with fp8
FastWeightLoad (4 weights/cycle/lane), so the PE does everything
(~16 us/rep) and the DVE is idle. Measured rel_err vs the f32 reference
is 1.2e-2 (tolerance 2e-2); all accumulation is f32 in PSUM, w and p stay
fp16.

Per batch b (P=128, NT=16 l-chunks, DC=8 d-chunks):
  s[lc]   = sum_c encB[c, lc].T @ w[c]      # 8 MMs -> PSUM [128, 1]
  p[:,lc] = exp(s[lc])                      # ACT, fp16
  ctx[c] += encA[lc, c].T @ p[:, lc]        # 8 MMs per lc -> PSUM [128,1]
  z      += p[:, lc].T @ ones               # 1 MM
Host divides ctx/z and reorders d = c*128 + p.

Sharding: data-parallel over batch B=32 across 8 NeuronCores (4/core).
"""

import sys

if "/opt/trn_rl_repo" not in sys.path:
    sys.path.insert(0, "/opt/trn_rl_repo")

import numpy as np
import ml_dtypes

import concourse.bacc as bacc
import concourse.mybir as mybir
import concourse.tile as tile
from concourse import bass_utils

F8NP = ml_dtypes.float8_e3m4

B, L, D = 32, 2048, 1024
NCORES = 8
B_LOC = B // NCORES          # 4 batches per core
P = 128                      # SBUF partitions
NT = L // P                  # 16 l-chunks per batch
DC = D // P                  # 8 d-chunks

ENCA_BUFS = 3                # encA batch slots [128, NT, D] f8 (2 MiB each);
                             # whole batch stays live through its ctx phase
                             # (c-outer loop), plus next-batch prefetch
ENCB_BUFS = 3                # encB batch slots [128, DC, L] f8 (2 MiB each)


def _build(reps=1):
    nc = bacc.Bacc("TRN2", target_bir_lowering=False, debug=False, num_devices=NCORES)
    f32 = mybir.dt.float32
    f16 = mybir.dt.float16
    f8 = mybir.dt.float8e3
    # Pre-tiled on host so each partition's whole batch line is contiguous
    # in HBM: encA row (b*128+p) = enc[b, k*128+p, :] for k=0..NT-1 concat;
    # encB row (b*128+p) = enc[b, :, c*128+p] for c=0..DC-1 concat.
    encA = nc.dram_tensor("encA", [B_LOC * P, NT * D], f8, kind="ExternalInput")
    encB = nc.dram_tensor("encB", [B_LOC * P, DC * L], f8, kind="ExternalInput")
    wcol = nc.dram_tensor("wcol", [P, DC], f16, kind="ExternalInput")
    ctxout = nc.dram_tensor("ctxout", [P, B_LOC * DC], f32, kind="ExternalOutput")
    zout = nc.dram_tensor("zout", [1, B_LOC], f32, kind="ExternalOutput")

    with tile.TileContext(nc) as tc:
        with (
            tc.tile_pool(name="const", bufs=1) as const_pool,
            tc.tile_pool(name="encb", bufs=ENCB_BUFS) as encb_pool,
            tc.tile_pool(name="enca", bufs=ENCA_BUFS) as enca_pool,
            tc.tile_pool(name="pp", bufs=2) as p_pool,
            tc.tile_pool(name="ctxsb", bufs=2) as ctxsb_pool,
            tc.tile_pool(name="zsb", bufs=2) as zsb_pool,
            tc.tile_pool(name="pss", bufs=2, space="PSUM") as ps_s,
            tc.tile_pool(name="psctx", bufs=2, space="PSUM") as ps_ctx,
            tc.tile_pool(name="psz", bufs=2, space="PSUM") as ps_z,
        ):
            wt = const_pool.tile([P, DC], f16)
            nc.sync.dma_start(wt[:], wcol[:])
            ones = const_pool.tile([P, 1], f16)
            nc.vector.memset(ones[:], 1.0)
            warm_in = const_pool.tile([1, 1], f32)
            nc.vector.memset(warm_in[:], 1.0)

            # Cold-start warmups: preload the ACT exp table and keep the PE
            # busy so the HAM clock gate reaches full rate.
            warm = zsb_pool.tile([1, 1], f16)
            nc.scalar.activation(
                warm[:], warm_in[:], mybir.ActivationFunctionType.Exp
            )
            wps = ps_z.tile([1, 1], f32)
            for i in range(48):
                nc.tensor.matmul(wps[:], ones[:], ones[:])

            for _ in range(reps):
                ctx_sb = ctxsb_pool.tile([P, B_LOC * DC], f32)
                z_sb = zsb_pool.tile([1, B_LOC], f32)
                for b in range(B_LOC):
                    # --- logits phase: encB chunks [128_d, L] ---
                    ebuf = encb_pool.tile([P, DC, L], f8)
                    nc.sync.dma_start(
                        ebuf[:],
                        encB[b * P : (b + 1) * P, :].rearrange(
                            "p (k l) -> p k l", k=DC
                        ),
                    )
                    ebufs = [ebuf[:, c, :] for c in range(DC)]
                    p_sb = p_pool.tile([P, NT], f16)
                    for t in range(NT):
                        s_ps = ps_s.tile([P, 1], f32)
                        for c in range(DC):
                            nc.tensor.matmul(
                                s_ps[:],
                                ebufs[c][:, t * P : (t + 1) * P],
                                wt[:, c : c + 1],
                                start=(c == 0),
                                stop=(c == DC - 1),
                                skip_group_check=True,
                            )
                        nc.scalar.activation(
                            p_sb[:, t : t + 1],
                            s_ps[:],
                            mybir.ActivationFunctionType.Exp,
                        )
                    # --- context phase: encA tiles [128_l, D] ---
                    ctx_ps = ps_ctx.tile([P, DC], f32)
                    z_ps = ps_z.tile([1, 1], f32)
                    abuf = enca_pool.tile([P, NT, D], f8)
                    nc.sync.dma_start(
                        abuf[:],
                        encA[b * P : (b + 1) * P, :].rearrange(
                            "p (k d) -> p k d", k=NT
                        ),
                    )
                    abufs = [abuf[:, t, :] for t in range(NT)]
                    # c-outer / t-inner: each PSUM column's accumulation group
                    # fully closes before the next column's start clears the
                    # bank's has_written bits (start is bank-wide, values
                    # persist).
                    for c in range(DC):
                        for t in range(NT):
                            nc.tensor.matmul(
                                ctx_ps[:, c : c + 1],
                                abufs[t][:, c * P : (c + 1) * P],
                                p_sb[:, t : t + 1],
                                start=(t == 0),
                                stop=(t == NT - 1),
                                skip_group_check=True,
                            )
                    for t in range(NT):
                        nc.tensor.matmul(
                            z_ps[:],
                            p_sb[:, t : t + 1],
                            ones[:],
                            start=(t == 0),
                            stop=(t == NT - 1),
                        )
                    nc.scalar.activation(
                        ctx_sb[:, b * DC : (b + 1) * DC],
                        ctx_ps[:],
                        mybir.ActivationFunctionType.Copy,
                    )
                    nc.scalar.activation(
                        z_sb[:, b : b + 1],
                        z_ps[:],
                        mybir.ActivationFunctionType.Copy,
                    )
                nc.scalar.dma_start(ctxout[:], ctx_sb[:])
                nc.scalar.dma_start(zout[:], z_sb[:])
    nc.compile()
    return nc


_NC = None


def _get_nc():
    global _NC
    if _NC is None:
        _NC = _build()
    return _NC


def _prep_inputs(encoder_output, W):
    enc32 = np.asarray(encoder_output, dtype=np.float32)
    enc8 = enc32.astype(F8NP)                                    # [B, L, D]
    # encA[b, p, k, d] = enc[b, k*128+p, d]  -> rows of 16 KiB, fully
    # contiguous per partition (descriptor-minimal DMA)
    encA = np.ascontiguousarray(
        enc8.reshape(B, NT, P, D).transpose(0, 2, 1, 3)
    )                                                            # [B, P, NT, D]
    # encB[b, p, c, l] = enc[b, l, c*128+p]
    encB = np.ascontiguousarray(
        enc8.transpose(0, 2, 1).reshape(B, DC, P, L).transpose(0, 2, 1, 3)
    )                                                            # [B, P, DC, L]
    w16 = np.asarray(W, dtype=np.float32)[:D, 0].astype(np.float16)
    wcol = np.ascontiguousarray(w16.reshape(DC, P).T)            # [128, 8]
    return encA, encB, wcol


def _run(nc, encA, encB, wcol, **kwargs):
    in_maps = [
        {
            "encA": np.ascontiguousarray(
                encA[i * B_LOC : (i + 1) * B_LOC].reshape(B_LOC * P, NT * D)
            ),
            "encB": np.ascontiguousarray(
                encB[i * B_LOC : (i + 1) * B_LOC].reshape(B_LOC * P, DC * L)
            ),
            "wcol": wcol,
        }
        for i in range(NCORES)
    ]
    res = bass_utils.run_bass_kernel_spmd(
        nc, in_maps, core_ids=list(range(NCORES)), **kwargs
    )
    out = np.empty((B, 1, D), np.float32)
    for i, r in enumerate(res.results):
        ctx = r["ctxout"]          # [128, B_LOC*DC]
        zz = r["zout"]             # [1, B_LOC]
        for b in range(B_LOC):
            m = ctx[:, b * DC : (b + 1) * DC]       # [128_p, DC_c]
            out[i * B_LOC + b, 0, :] = m.T.reshape(D) / zz[0, b]
    return out, res


def kernel(encoder_output, decoder_hidden=None, W=None, b=None):
    encA, encB, wcol = _prep_inputs(encoder_output, W)
    out, _ = _run(_get_nc(), encA, encB, wcol)
    return out
